# revision 3
# baseline (speedup 1.0000x reference)
"""CANINE self-attention (relative_key_query) TRN2 Bass kernel, 8-core SPMD, v2.

Sharding: data-parallel over batch (4) x tensor-parallel over heads (12 -> 2
groups of 6). Core c handles batch c//2, heads [6*(c%2), 6*(c%2)+6).

v2 vs v1 (~280us): the scores path runs fp8e4m3 with DoubleRow perf mode
(2 PE rows/cycle) for S1 qk, the Dq/Dk pos-window matmuls, the Toeplitz-skew
A-transposes, and the k-term accumulate; the k-term is matmul-accumulated
into PSUM (no DVE add) and EXP reads PSUM directly; psum->fp8 casts are
cost-balanced across Vector/Scalar/GpSimd; input DMAs are split per-k-chunk
so projections start early; PSUM is planned as 8 single-bank tiles; phase A
of head-pair hp+1 is interleaved with phase B of hp so the PE never drains;
output is DMA'd per (head, l-tile).

Numerics: projections bf16 x bf16 -> f32; q/k quantized to fp8 post-bias
(feeds S1 and the pos path; ~1e-2 rel err total); probs and V stay bf16
(fp8 there costs ~1.7e-2 each). DoubleRow folds the contraction dim as
[K/2, 2, .] with the 2 k-subtiles on the free axis.
"""

import sys

sys.path.insert(0, "/opt/trn_rl_repo")

from contextlib import ExitStack

import ml_dtypes
import numpy as np

import concourse.bass as bass
import concourse.tile as tile
from concourse import bacc, mybir
from concourse.bass_utils import run_bass_kernel_spmd
from concourse.masks import make_identity

BF16 = ml_dtypes.bfloat16
B, L, H, NH, HD = 4, 1024, 768, 12, 64
MAX_POS = 1024
NCORES = 8
HPC = NH // 2          # heads per core = 6
JP = 2048              # padded relative-index axis (>= 2047)
W = 1152               # per-tile j-window width (= 512+512+128 >= 1151)
ROWB = 128 * W         # flat scratch elements per l/r tile
DR = mybir.MatmulPerfMode.DoubleRow

_nc_cache = {}


def _build_nc():
    nc = bacc.Bacc(
        "TRN2",
        target_bir_lowering=False,
        debug=False,
        enable_asserts=True,
        num_devices=NCORES,
    )
    f32 = mybir.dt.float32
    bf16 = mybir.dt.bfloat16
    fp8 = mybir.dt.float8e4

    xfT = nc.dram_tensor("xfT", [H, L], bf16, kind="ExternalInput")
    xtT = nc.dram_tensor("xtT", [H, L], bf16, kind="ExternalInput")
    wqT = nc.dram_tensor("wqT", [H, HPC * HD], bf16, kind="ExternalInput")
    wkT = nc.dram_tensor("wkT", [H, HPC * HD], bf16, kind="ExternalInput")
    wvT = nc.dram_tensor("wvT", [H, HPC * HD], bf16, kind="ExternalInput")
    bqp = nc.dram_tensor("bqp", [128, 3], f32, kind="ExternalInput")
    bkp = nc.dram_tensor("bkp", [128, 3], f32, kind="ExternalInput")
    bvr = nc.dram_tensor("bvr", [1, HPC * HD], f32, kind="ExternalInput")
    E8qd = nc.dram_tensor("E8qd", [32, 2, JP], fp8, kind="ExternalInput")
    E8kd = nc.dram_tensor("E8kd", [32, 2, JP], fp8, kind="ExternalInput")
    Ifd = nc.dram_tensor("Ifd", [64, 2, 128], fp8, kind="ExternalInput")
    out = nc.dram_tensor("out", [L, HPC * HD], f32, kind="ExternalOutput")

    Ident = mybir.ActivationFunctionType.Identity
    Exp = mybir.ActivationFunctionType.Exp
    add = mybir.AluOpType.add
    mult = mybir.AluOpType.mult

    with tile.TileContext(nc) as tc, ExitStack() as ctx:
        const = ctx.enter_context(tc.tile_pool(name="const", bufs=1))
        stg_pool = ctx.enter_context(tc.tile_pool(name="stg", bufs=6))
        af_pool = ctx.enter_context(tc.tile_pool(name="afp", bufs=2))
        bt_pool = ctx.enter_context(tc.tile_pool(name="btp", bufs=4))
        ept_pool = ctx.enter_context(tc.tile_pool(name="eptp", bufs=4))
        ctxt_pool = ctx.enter_context(tc.tile_pool(name="ctxtp", bufs=2))
        zr_pool = ctx.enter_context(tc.tile_pool(name="zrp", bufs=4))
        psA = ctx.enter_context(tc.tile_pool(name="psA", bufs=4, space="PSUM"))
        ps1 = ctx.enter_context(tc.tile_pool(name="ps1", bufs=2, space="PSUM"))
        pct = ctx.enter_context(tc.tile_pool(name="pct", bufs=2, space="PSUM"))
        dram_pool = ctx.enter_context(tc.tile_pool(name="scr", bufs=2, space="DRAM"))

        # ---- elementwise engine cost balancer (ns estimates) ----
        est = {"v": 0.0, "s": 0.0, "g": 0.0}
        RATE = {"v": 1.05, "s": 0.84, "g": 1.39}
        INIT = {"v": 130.0, "s": 145.0, "g": 200.0}

        def pick(n, engines):
            e = min(engines, key=lambda e: est[e] + n * RATE[e] + INIT[e])
            est[e] += n * RATE[e] + INIT[e]
            return e

        def cast(dst, src, n, engines=("v", "s")):
            e = pick(n, engines)
            if e == "v":
                nc.vector.tensor_copy(dst, src)
            elif e == "s":
                nc.scalar.copy(dst, src)
            else:
                nc.gpsimd.tensor_copy(dst, src)

        # ---- constant loads, split per k-chunk for early start ----
        wq_sb = const.tile([128, 6, HPC * HD], bf16)
        wk_sb = const.tile([128, 6, HPC * HD], bf16)
        wv_sb = const.tile([128, 6, HPC * HD], bf16)
        xf_sb = const.tile([128, 6, L], bf16)
        xt_sb = const.tile([128, 6, L], bf16)
        wq_ap = wqT.ap().rearrange("(t p) d -> p t d", p=128)
        wk_ap = wkT.ap().rearrange("(t p) d -> p t d", p=128)
        wv_ap = wvT.ap().rearrange("(t p) d -> p t d", p=128)
        xf_ap = xfT.ap().rearrange("(t p) l -> p t l", p=128)
        xt_ap = xtT.ap().rearrange("(t p) l -> p t l", p=128)
        for ki in range(6):
            nc.sync.dma_start(wq_sb[:, ki, :], wq_ap[:, ki, :])
            nc.sync.dma_start(xf_sb[:, ki, :], xf_ap[:, ki, :])
        bq_sb = const.tile([128, 3], f32)
        nc.sync.dma_start(bq_sb, bqp.ap())
        bk_sb = const.tile([128, 3], f32)
        nc.sync.dma_start(bk_sb, bkp.ap())
        for ki in range(6):
            nc.sync.dma_start(wk_sb[:, ki, :], wk_ap[:, ki, :])
            nc.sync.dma_start(xt_sb[:, ki, :], xt_ap[:, ki, :])
        for ki in range(6):
            nc.sync.dma_start(wv_sb[:, ki, :], wv_ap[:, ki, :])
        e8q_sb = const.tile([32, 2, JP], fp8)
        nc.sync.dma_start(e8q_sb, E8qd.ap())
        e8k_sb = const.tile([32, 2, JP], fp8)
        nc.sync.dma_start(e8k_sb, E8kd.ap())
        if_sb = const.tile([64, 2, 128], fp8)
        nc.sync.dma_start(if_sb, Ifd.ap())
        bv_bc = const.tile([128, HPC * HD], f32)
        bv_ap = bvr.ap()
        nc.gpsimd.dma_start(
            bv_bc,
            bass.AP(tensor=bv_ap.tensor, offset=bv_ap.offset,
                    ap=[[0, 128]] + bv_ap.ap[1:]),
        )
        ident_bf = const.tile([128, 128], bf16)
        make_identity(nc, ident_bf)

        qt8 = const.tile([128, 3, L], fp8)
        kt8 = const.tile([128, 3, L], fp8)
        qf8 = const.tile([32, 3, 2, 2, L], fp8)
        kf8 = const.tile([32, 3, 2, 2, L], fp8)
        vaug = const.tile([128, 8, HPC * 65], bf16)
        out_sb = const.tile([128, 8, HPC * HD], f32)

        # ---- Q/K projections (bf16 matmuls, fp8 out post-bias) + folds ----
        for dt in range(3):
            for w_sb, x_sb, b_sb, dst8 in (
                (wq_sb, xf_sb, bq_sb, qt8),
                (wk_sb, xt_sb, bk_sb, kt8),
            ):
                for nh in range(2):
                    ps = psA.tile([128, 512], f32, tag="pa", name=f"pj{dt}{nh}")
                    for ki in range(6):
                        nc.tensor.matmul(
                            ps,
                            lhsT=w_sb[:, ki, dt * 128:(dt + 1) * 128],
                            rhs=x_sb[:, ki, nh * 512:(nh + 1) * 512],
                            start=(ki == 0),
                            stop=(ki == 5),
                        )
                    nc.scalar.activation(
                        out=dst8[:, dt, nh * 512:(nh + 1) * 512],
                        in_=ps,
                        func=Ident,
                        bias=b_sb[:, dt:dt + 1],
                        scale=1.0,
                    )
                    est["s"] += 512 * RATE["s"] + INIT["s"]
            for hi in range(2):
                for kt in range(2):
                    p0 = 64 * hi + 32 * kt
                    nc.sync.dma_start(qf8[0:32, dt, hi, kt, :],
                                      qt8[p0:p0 + 32, dt, :])
                    nc.sync.dma_start(kf8[0:32, dt, hi, kt, :],
                                      kt8[p0:p0 + 32, dt, :])

        # ---- V projections (emitted interleaved with phase A of hp=0) ----
        def vproj_units():
            for rt in range(8):
                ps = psA.tile([128, 512], f32, tag="pa", name=f"pv{rt}")
                for ki in range(6):
                    nc.tensor.matmul(
                        ps[:, 0:HPC * HD],
                        lhsT=xt_sb[:, ki, rt * 128:(rt + 1) * 128],
                        rhs=wv_sb[:, ki, :],
                        start=(ki == 0),
                        stop=(ki == 5),
                    )
                nc.vector.tensor_tensor(
                    vaug[:, rt].rearrange("p (h e) -> p h e", e=65)[:, :, 0:HD],
                    ps[:, 0:HPC * HD].rearrange("p (h d) -> p h d", d=HD),
                    bv_bc.rearrange("p (h d) -> p h d", d=HD),
                    add,
                )
                est["v"] += 384 * RATE["v"] + INIT["v"]
                if rt == 7:
                    nc.vector.memset(
                        vaug.rearrange("p r (h e) -> p r h e", e=65)[:, :, :, 64:65],
                        1.0,
                    )
                yield

        scr = {}

        def phase_a(hp):
            for side, e_sb, fold8 in (("q", e8q_sb, qf8), ("k", e8k_sb, kf8)):
                for hi in range(2):
                    scr[(hp, side, hi)] = dram_pool.tile(
                        [8 * ROWB], fp8, tag=f"{side}{hi}", name=f"scr{side}{hi}"
                    )
                for lt in range(8):
                    w0 = 896 - lt * 128
                    for hi in range(2):
                        stg = stg_pool.tile([128, W], fp8, tag="stg",
                                            name=f"stg{hp}{side}{lt}{hi}")
                        for c, cw in ((0, 512), (512, 512), (1024, 128)):
                            ps = psA.tile([128, 512], f32, tag="pa",
                                          name=f"pA{hp}{side}{lt}{hi}{c}")
                            nc.tensor.matmul(
                                ps[:, 0:cw],
                                lhsT=fold8[0:32, hp, hi, :, lt * 128:(lt + 1) * 128],
                                rhs=e_sb[0:32, :, w0 + c:w0 + c + cw],
                                perf_mode=DR,
                                start=True,
                                stop=True,
                            )
                            cast(stg[:, c:c + cw], ps[:, 0:cw], cw)
                        nc.sync.dma_start(
                            scr[(hp, side, hi)][lt * ROWB:(lt + 1) * ROWB]
                            .rearrange("(p w) -> p w", w=W),
                            stg,
                        )
                        yield

        def skew_read(scrt, blk):
            base = blk * ROWB + 127
            return (
                scrt[base:base + 2 * 64 * (W - 1)]
                .rearrange("(kt p w) -> p kt w", kt=2, w=W - 1)[:, :, 0:L]
            )

        def phase_b(hp):
            for hi in range(2):
                h = 2 * hp + hi
                af = af_pool.tile([64, 8, 2, L], fp8, tag="af", name=f"af{h}")
                for lt in range(8):
                    nc.sync.dma_start(af[:, lt, :, :],
                                      skew_read(scr[(hp, "q", hi)], lt))
                pcts = [
                    pct.tile([128, 512], f32, tag="pc", name=f"pv{h}{nh}")
                    for nh in range(2)
                ]
                yield
                pending_pv = []
                for rt in range(8):
                    bt = bt_pool.tile([64, 2, L], fp8, tag="bt", name=f"bt{h}{rt}")
                    nc.sync.dma_start(bt, skew_read(scr[(hp, "k", hi)], rt))
                    ept = ept_pool.tile([128, L], bf16, tag="ept",
                                        name=f"ept{h}{rt}")
                    for nh in range(2):
                        pst = ps1.tile([128, 512], f32, tag="pst",
                                       name=f"pst{h}{rt}{nh}")
                        nc.tensor.matmul(
                            pst,
                            lhsT=kf8[0:32, hp, hi, :, rt * 128:(rt + 1) * 128],
                            rhs=qf8[0:32, hp, hi, :, nh * 512:(nh + 1) * 512],
                            perf_mode=DR,
                            start=True,
                            stop=False,
                            skip_group_check=True,
                        )
                        for j in range(4):
                            lt = 4 * nh + j
                            nc.tensor.matmul(
                                pst[:, j * 128:(j + 1) * 128],
                                lhsT=af[:, lt, :, rt * 128:(rt + 1) * 128],
                                rhs=if_sb,
                                perf_mode=DR,
                                start=False,
                                stop=False,
                                skip_group_check=True,
                            )
                        if nh == 0:
                            # drain previous rt's PV matmuls mid-group so they
                            # never wait on a just-issued EXP
                            for pv_fn in pending_pv:
                                pv_fn()
                            pending_pv = []
                        nc.tensor.matmul(
                            pst,
                            lhsT=if_sb,
                            rhs=bt[:, :, nh * 512:(nh + 1) * 512],
                            perf_mode=DR,
                            start=False,
                            stop=True,
                            skip_group_check=True,
                        )
                        nc.scalar.activation(
                            out=ept[:, nh * 512:(nh + 1) * 512],
                            in_=pst,
                            func=Exp,
                            scale=0.125,
                        )
                        est["s"] += 512 * RATE["s"] + INIT["s"]

                        def pv_fn(rt=rt, nh=nh, ept=ept):
                            nc.tensor.matmul(
                                pcts[nh][0:65, :],
                                lhsT=vaug[:, rt, h * 65:h * 65 + 65],
                                rhs=ept[:, nh * 512:(nh + 1) * 512],
                                start=(rt == 0),
                                stop=(rt == 7),
                            )

                        pending_pv.append(pv_fn)
                    yield
                for pv_fn in pending_pv:
                    pv_fn()
                ctxt = ctxt_pool.tile([128, L], bf16, tag="ctxt", name=f"ctx{h}")
                for nh in range(2):
                    cast(ctxt[0:65, nh * 512:(nh + 1) * 512], pcts[nh][0:65, :],
                         512, engines=("v", "s"))
                yield
                for lt in range(8):
                    ctr = pct.tile([128, 512], bf16, tag="pc", name=f"ctr{h}{lt}")
                    nc.tensor.matmul(
                        ctr[:, 0:65],
                        lhsT=ctxt[0:65, lt * 128:(lt + 1) * 128],
                        rhs=ident_bf[0:65, 0:65],
                        is_transpose=True,
                    )
                    zr = zr_pool.tile([128, 1], f32, tag="zr", name=f"zr{h}{lt}")
                    nc.vector.reciprocal(zr, ctr[:, 64:65])
                    nc.vector.tensor_tensor(
                        out_sb[:, lt, h * HD:(h + 1) * HD],
                        ctr[:, 0:HD],
                        zr.to_broadcast([128, HD]),
                        mult,
                    )
                    est["v"] += 64 * RATE["v"] + 2 * INIT["v"]
                    nc.sync.dma_start(
                        out.ap()[lt * 128:(lt + 1) * 128, h * HD:(h + 1) * HD],
                        out_sb[:, lt, h * HD:(h + 1) * HD],
                    )
                yield

        # ---- drive: A(0) ∥ V-proj, then B(hp) ∥ A(hp+1) ----
        def drain(g, n=None):
            cnt = 0
            while n is None or cnt < n:
                try:
                    next(g)
                except StopIteration:
                    return False
                cnt += 1
            return True

        vg = vproj_units()
        a_cur = phase_a(0)
        i = 0
        while True:
            alive_a = drain(a_cur, 1)
            if i % 4 == 0:
                drain(vg, 1)
            i += 1
            if not alive_a:
                break
        drain(vg)

        for hp in range(3):
            b = phase_b(hp)
            a_next = phase_a(hp + 1) if hp + 1 < 3 else None
            while True:
                alive_b = drain(b, 1)
                if a_next is not None:
                    drain(a_next, 2)
                if not alive_b:
                    break
            if a_next is not None:
                drain(a_next)

    nc.compile()
    return nc


def get_nc():
    if "nc" not in _nc_cache:
        _nc_cache["nc"] = _build_nc()
    return _nc_cache["nc"]


def make_in_maps(from_tensor, to_tensor, Wq, bq, Wk, bk, Wv, bv, dist_emb):
    F8NP = mybir.dt.np(mybir.dt.float8e4)
    E = np.asarray(dist_emb, np.float32)
    Epad = np.zeros((JP, HD), np.float32)
    Epad[: 2 * MAX_POS - 1] = E
    EFpad = np.zeros((JP, HD), np.float32)
    EFpad[: 2 * MAX_POS - 1] = E[::-1]
    # E8[p, kt, j] = Epad[j, 32 kt + p]
    E8k = np.ascontiguousarray(
        Epad.T.reshape(2, 32, JP).transpose(1, 0, 2)
    ).astype(F8NP)
    E8q = np.ascontiguousarray(
        EFpad.T.reshape(2, 32, JP).transpose(1, 0, 2)
    ).astype(F8NP)
    If = np.zeros((64, 2, 128), np.float32)
    for kt in range(2):
        If[np.arange(64), kt, 64 * kt + np.arange(64)] = 1.0
    If = If.astype(F8NP)

    in_maps = []
    for c in range(NCORES):
        b = c // 2
        h0 = (c % 2) * HPC
        sl = slice(h0 * HD, (h0 + HPC) * HD)
        in_maps.append(
            {
                "xfT": np.ascontiguousarray(np.asarray(from_tensor[b], np.float32).T).astype(BF16),
                "xtT": np.ascontiguousarray(np.asarray(to_tensor[b], np.float32).T).astype(BF16),
                "wqT": np.ascontiguousarray(np.asarray(Wq, np.float32)[sl].T).astype(BF16),
                "wkT": np.ascontiguousarray(np.asarray(Wk, np.float32)[sl].T).astype(BF16),
                "wvT": np.ascontiguousarray(np.asarray(Wv, np.float32)[sl].T).astype(BF16),
                "bqp": np.ascontiguousarray(np.asarray(bq, np.float32)[sl].reshape(3, 128).T),
                "bkp": np.ascontiguousarray(np.asarray(bk, np.float32)[sl].reshape(3, 128).T),
                "bvr": np.asarray(bv, np.float32)[sl].reshape(1, HPC * HD).copy(),
                "E8qd": E8q,
                "E8kd": E8k,
                "Ifd": If,
            }
        )
    return in_maps


def assemble(results):
    full = np.zeros((B, L, H), np.float32)
    for c in range(NCORES):
        b = c // 2
        h0 = (c % 2) * HPC
        full[b, :, h0 * HD:(h0 + HPC) * HD] = results[c]["out"]
    return full


def kernel(**inputs):
    import os
    os.environ["BASS_NEVER_TRACE"] = "1"  # NTFF hook is absent in grading env
    in_maps = make_in_maps(**inputs)
    nc = get_nc()
    res = run_bass_kernel_spmd(nc, in_maps, core_ids=list(range(NCORES)))
    return assemble(res.results)


if __name__ == "__main__":
    rng = np.random.default_rng(0)
    ins = {
        "from_tensor": rng.standard_normal((B, L, H), dtype=np.float32),
        "to_tensor": rng.standard_normal((B, L, H), dtype=np.float32),
        "Wq": rng.standard_normal((H, H), dtype=np.float32) * 0.02,
        "bq": rng.standard_normal((H,), dtype=np.float32) * 0.02,
        "Wk": rng.standard_normal((H, H), dtype=np.float32) * 0.02,
        "bk": rng.standard_normal((H,), dtype=np.float32) * 0.02,
        "Wv": rng.standard_normal((H, H), dtype=np.float32) * 0.02,
        "bv": rng.standard_normal((H,), dtype=np.float32) * 0.02,
        "dist_emb": rng.standard_normal((2 * MAX_POS - 1, HD), dtype=np.float32) * 0.02,
    }
    out = kernel(**ins)
    print("ran", out.shape, out.dtype)


# revision 5
# speedup vs baseline: 1.5603x; 1.5603x over previous
"""CANINE self-attention (relative_key_query) TRN2 Bass kernel, 8-core SPMD, v3.

Sharding: data-parallel over batch (4) x tensor-parallel over heads (12 -> 2
groups of 6). Core c handles batch c//2, heads [6*(c%2), 6*(c%2)+6).

Structure per head (same math as v1): QT/KT = (x@W.T+b).T in [d, l] layout;
scores built transposed S.T[r, l] = qk.T + A.T + B.T where the Toeplitz skew
of the relative-position terms is realized via a DRAM fp8 scratch written
[128, 1152]-row-major and re-read with row stride 1151 (q side additionally
transposed into place by fp8 identity-matmuls); softmax skips max-subtraction
and normalizes after PV via a ones-column in V.

v3 systems changes vs v1 (~280us):
 - input DMAs split per-k-chunk and ordered so Q-projection starts ~1.5us in
 - PSUM planned as 8 single-bank tiles: 4 chunk bufs (projections + phase-A
   windows), 2 score-half bufs, 2 PV/ctx-transpose bufs
 - psum->fp8 window casts cost-balanced across Vector AND Scalar engines
 - S.T is built in [128, 512] nh-halves so softmax pipelines at half-tiles
 - phase A of head-pair hp+1 is emission-interleaved with phase B of hp so
   the PE queue always has work; PV matmuls are deferred one rt-group so
   they never wait on a just-issued EXP
 - ctx transpose path runs bf16 (1 cycle/row) instead of f32 (2)
 - output DMA'd per (head, l-tile) instead of all-at-end

All matmuls are plain bf16/fp8 (no fp8 DoubleRow: measured on hw it gives no
per-row gain and the whole region clocks down ~2x).
"""

import sys

sys.path.insert(0, "/opt/trn_rl_repo")

from contextlib import ExitStack

import ml_dtypes
import numpy as np

import concourse.bass as bass
import concourse.tile as tile
from concourse import bacc, mybir
from concourse.bass_utils import run_bass_kernel_spmd
from concourse.masks import make_identity

BF16 = ml_dtypes.bfloat16
B, L, H, NH, HD = 4, 1024, 768, 12, 64
MAX_POS = 1024
NCORES = 8
HPC = NH // 2          # heads per core = 6
JP = 2048              # padded relative-index axis (>= 2047)
W = 1152               # per-tile j-window width (= 512+512+128 >= 1151)
ROWB = 128 * W         # flat scratch elements per l/r tile

_nc_cache = {}


def _build_nc():
    nc = bacc.Bacc(
        "TRN2",
        target_bir_lowering=False,
        debug=False,
        enable_asserts=True,
        num_devices=NCORES,
    )
    f32 = mybir.dt.float32
    bf16 = mybir.dt.bfloat16
    fp8 = mybir.dt.float8e4

    xfT = nc.dram_tensor("xfT", [H, L], bf16, kind="ExternalInput")
    xtT = nc.dram_tensor("xtT", [H, L], bf16, kind="ExternalInput")
    wqT = nc.dram_tensor("wqT", [H, HPC * HD], bf16, kind="ExternalInput")
    wkT = nc.dram_tensor("wkT", [H, HPC * HD], bf16, kind="ExternalInput")
    wvT = nc.dram_tensor("wvT", [H, HPC * HD], bf16, kind="ExternalInput")
    bqp = nc.dram_tensor("bqp", [128, 3], f32, kind="ExternalInput")
    bkp = nc.dram_tensor("bkp", [128, 3], f32, kind="ExternalInput")
    bvr = nc.dram_tensor("bvr", [1, HPC * HD], f32, kind="ExternalInput")
    ETd = nc.dram_tensor("ETd", [128, JP], bf16, kind="ExternalInput")
    EFTd = nc.dram_tensor("EFTd", [128, JP], bf16, kind="ExternalInput")
    out = nc.dram_tensor("out", [L, HPC * HD], f32, kind="ExternalOutput")

    Ident = mybir.ActivationFunctionType.Identity
    Exp = mybir.ActivationFunctionType.Exp
    add = mybir.AluOpType.add
    mult = mybir.AluOpType.mult

    with tile.TileContext(nc) as tc, ExitStack() as ctx:
        const = ctx.enter_context(tc.tile_pool(name="const", bufs=1))
        stg_pool = ctx.enter_context(tc.tile_pool(name="stg", bufs=6))
        af_pool = ctx.enter_context(tc.tile_pool(name="afp", bufs=2))
        bt_pool = ctx.enter_context(tc.tile_pool(name="btp", bufs=4))
        ssb_pool = ctx.enter_context(tc.tile_pool(name="ssbp", bufs=4))
        ept_pool = ctx.enter_context(tc.tile_pool(name="eptp", bufs=4))
        ctxt_pool = ctx.enter_context(tc.tile_pool(name="ctxtp", bufs=2))
        zr_pool = ctx.enter_context(tc.tile_pool(name="zrp", bufs=4))
        psA = ctx.enter_context(tc.tile_pool(name="psA", bufs=4, space="PSUM"))
        ps1 = ctx.enter_context(tc.tile_pool(name="ps1", bufs=2, space="PSUM"))
        pct = ctx.enter_context(tc.tile_pool(name="pct", bufs=2, space="PSUM"))
        dram_pool = ctx.enter_context(tc.tile_pool(name="scr", bufs=2, space="DRAM"))

        # ---- elementwise engine cost balancer (ns estimates) ----
        est = {"v": 0.0, "s": 0.0}
        RATE = {"v": 1.05, "s": 0.84}
        INIT = {"v": 130.0, "s": 145.0}

        def pick(n, engines):
            e = min(engines, key=lambda e: est[e] + n * RATE[e] + INIT[e])
            est[e] += n * RATE[e] + INIT[e]
            return e

        def cast(dst, src, n, engines=("v", "s")):
            e = pick(n, engines)
            if e == "v":
                nc.vector.tensor_copy(dst, src)
            else:
                nc.scalar.copy(dst, src)

        # ---- constant loads, split per k-chunk for early start ----
        wq_sb = const.tile([128, 6, HPC * HD], bf16)
        wk_sb = const.tile([128, 6, HPC * HD], bf16)
        wv_sb = const.tile([128, 6, HPC * HD], bf16)
        xf_sb = const.tile([128, 6, L], bf16)
        xt_sb = const.tile([128, 6, L], bf16)
        wq_ap = wqT.ap().rearrange("(t p) d -> p t d", p=128)
        wk_ap = wkT.ap().rearrange("(t p) d -> p t d", p=128)
        wv_ap = wvT.ap().rearrange("(t p) d -> p t d", p=128)
        xf_ap = xfT.ap().rearrange("(t p) l -> p t l", p=128)
        xt_ap = xtT.ap().rearrange("(t p) l -> p t l", p=128)
        for ki in range(6):
            nc.sync.dma_start(wq_sb[:, ki, :], wq_ap[:, ki, :])
            nc.sync.dma_start(xf_sb[:, ki, :], xf_ap[:, ki, :])
        bq_sb = const.tile([128, 3], f32)
        nc.sync.dma_start(bq_sb, bqp.ap())
        bk_sb = const.tile([128, 3], f32)
        nc.sync.dma_start(bk_sb, bkp.ap())
        for ki in range(6):
            nc.sync.dma_start(wk_sb[:, ki, :], wk_ap[:, ki, :])
            nc.sync.dma_start(xt_sb[:, ki, :], xt_ap[:, ki, :])
        for ki in range(6):
            nc.sync.dma_start(wv_sb[:, ki, :], wv_ap[:, ki, :])
        et_sb = const.tile([128, JP], bf16)
        nc.sync.dma_start(et_sb, ETd.ap())
        eft_sb = const.tile([128, JP], bf16)
        nc.sync.dma_start(eft_sb, EFTd.ap())
        bv_bc = const.tile([128, HPC * HD], f32)
        bv_ap = bvr.ap()
        nc.gpsimd.dma_start(
            bv_bc,
            bass.AP(tensor=bv_ap.tensor, offset=bv_ap.offset,
                    ap=[[0, 128]] + bv_ap.ap[1:]),
        )
        ident_bf = const.tile([128, 128], bf16)
        make_identity(nc, ident_bf)
        ident_f8 = const.tile([128, 128], fp8)
        make_identity(nc, ident_f8)

        qt_sb = const.tile([128, 3, L], bf16)
        kt_sb = const.tile([128, 3, L], bf16)
        vaug = const.tile([128, 8, HPC * 65], bf16)
        out_sb = const.tile([128, 8, HPC * HD], f32)

        # ---- Q/K projections (bf16 matmuls, bf16 out post-bias) ----
        for dt in range(3):
            for w_sb, x_sb, b_sb, dst in (
                (wq_sb, xf_sb, bq_sb, qt_sb),
                (wk_sb, xt_sb, bk_sb, kt_sb),
            ):
                for nh in range(2):
                    ps = psA.tile([128, 512], f32, tag="pa", name=f"pj{dt}{nh}")
                    for ki in range(6):
                        nc.tensor.matmul(
                            ps,
                            lhsT=w_sb[:, ki, dt * 128:(dt + 1) * 128],
                            rhs=x_sb[:, ki, nh * 512:(nh + 1) * 512],
                            start=(ki == 0),
                            stop=(ki == 5),
                        )
                    nc.scalar.activation(
                        out=dst[:, dt, nh * 512:(nh + 1) * 512],
                        in_=ps,
                        func=Ident,
                        bias=b_sb[:, dt:dt + 1],
                        scale=1.0,
                    )
                    est["s"] += 512 * RATE["s"] + INIT["s"]

        # ---- V projections (emitted interleaved with phase A of hp=0) ----
        def vproj_units():
            for rt in range(8):
                ps = psA.tile([128, 512], f32, tag="pa", name=f"pv{rt}")
                for ki in range(6):
                    nc.tensor.matmul(
                        ps[:, 0:HPC * HD],
                        lhsT=xt_sb[:, ki, rt * 128:(rt + 1) * 128],
                        rhs=wv_sb[:, ki, :],
                        start=(ki == 0),
                        stop=(ki == 5),
                    )
                nc.vector.tensor_tensor(
                    vaug[:, rt].rearrange("p (h e) -> p h e", e=65)[:, :, 0:HD],
                    ps[:, 0:HPC * HD].rearrange("p (h d) -> p h d", d=HD),
                    bv_bc.rearrange("p (h d) -> p h d", d=HD),
                    add,
                )
                est["v"] += 384 * RATE["v"] + INIT["v"]
                if rt == 7:
                    nc.vector.memset(
                        vaug.rearrange("p r (h e) -> p r h e", e=65)[:, :, :, 64:65],
                        1.0,
                    )
                yield

        scr = {}

        def phase_a(hp):
            for side, e_sb, qk_sb in (("q", eft_sb, qt_sb), ("k", et_sb, kt_sb)):
                for hi in range(2):
                    scr[(hp, side, hi)] = dram_pool.tile(
                        [8 * ROWB], fp8, tag=f"{side}{hi}", name=f"scr{side}{hi}"
                    )
                for lt in range(8):
                    w0 = 896 - lt * 128
                    for hi in range(2):
                        rh = slice(64 * hi, 64 * hi + 64)
                        stg = stg_pool.tile([128, W], fp8, tag="stg",
                                            name=f"stg{hp}{side}{lt}{hi}")
                        for c, cw in ((0, 512), (512, 512), (1024, 128)):
                            ps = psA.tile([128, 512], f32, tag="pa",
                                          name=f"pA{hp}{side}{lt}{hi}{c}")
                            nc.tensor.matmul(
                                ps[:, 0:cw],
                                lhsT=qk_sb[rh, hp, lt * 128:(lt + 1) * 128],
                                rhs=e_sb[rh, w0 + c:w0 + c + cw],
                                start=True,
                                stop=True,
                            )
                            cast(stg[:, c:c + cw], ps[:, 0:cw], cw)
                        nc.sync.dma_start(
                            scr[(hp, side, hi)][lt * ROWB:(lt + 1) * ROWB]
                            .rearrange("(p w) -> p w", w=W),
                            stg,
                        )
                        yield

        def skew_read(scrt, blk):
            base = blk * ROWB + 127
            return (
                scrt[base:base + 128 * (W - 1)]
                .rearrange("(p w) -> p w", w=W - 1)[:, 0:L]
            )

        def phase_b(hp):
            for hi in range(2):
                h = 2 * hp + hi
                rh = slice(64 * hi, 64 * hi + 64)
                af = af_pool.tile([128, 8, L], fp8, tag="af", name=f"af{h}")
                for lt in range(8):
                    nc.sync.dma_start(af[:, lt, :],
                                      skew_read(scr[(hp, "q", hi)], lt))
                pcts = [
                    pct.tile([128, 512], f32, tag="pc", name=f"pv{h}{nh}")
                    for nh in range(2)
                ]
                yield
                pending_pv = []
                for rt in range(8):
                    bt = bt_pool.tile([128, L], fp8, tag="bt", name=f"bt{h}{rt}")
                    nc.sync.dma_start(bt, skew_read(scr[(hp, "k", hi)], rt))
                    s_sb = ssb_pool.tile([128, L], bf16, tag="ssb",
                                         name=f"ssb{h}{rt}")
                    ept = ept_pool.tile([128, L], bf16, tag="ept",
                                        name=f"ept{h}{rt}")
                    for nh in range(2):
                        pst = ps1.tile([128, 512], f32, tag="pst",
                                       name=f"pst{h}{rt}{nh}")
                        nc.tensor.matmul(
                            pst,
                            lhsT=kt_sb[rh, hp, rt * 128:(rt + 1) * 128],
                            rhs=qt_sb[rh, hp, nh * 512:(nh + 1) * 512],
                            start=True,
                            stop=False,
                            skip_group_check=True,
                        )
                        for j in range(4):
                            lt = 4 * nh + j
                            nc.tensor.matmul(
                                pst[:, j * 128:(j + 1) * 128],
                                lhsT=af[:, lt, rt * 128:(rt + 1) * 128],
                                rhs=ident_f8,
                                start=False,
                                stop=True,
                                skip_group_check=True,
                            )
                        if nh == 0:
                            # drain previous rt's PV matmuls mid-group so they
                            # never wait on a just-issued EXP
                            for pv_fn in pending_pv:
                                pv_fn()
                            pending_pv = []
                        nc.vector.tensor_tensor(
                            s_sb[:, nh * 512:(nh + 1) * 512],
                            pst,
                            bt[:, nh * 512:(nh + 1) * 512],
                            add,
                        )
                        est["v"] += 512 * RATE["v"] + INIT["v"]
                        nc.scalar.activation(
                            out=ept[:, nh * 512:(nh + 1) * 512],
                            in_=s_sb[:, nh * 512:(nh + 1) * 512],
                            func=Exp,
                            scale=0.125,
                        )
                        est["s"] += 512 * RATE["s"] + INIT["s"]

                        def pv_fn(rt=rt, nh=nh, ept=ept):
                            nc.tensor.matmul(
                                pcts[nh][0:65, :],
                                lhsT=vaug[:, rt, h * 65:h * 65 + 65],
                                rhs=ept[:, nh * 512:(nh + 1) * 512],
                                start=(rt == 0),
                                stop=(rt == 7),
                            )

                        pending_pv.append(pv_fn)
                    yield
                for pv_fn in pending_pv:
                    pv_fn()
                ctxt = ctxt_pool.tile([128, L], bf16, tag="ctxt", name=f"ctx{h}")
                for nh in range(2):
                    cast(ctxt[0:65, nh * 512:(nh + 1) * 512], pcts[nh][0:65, :],
                         512)
                yield
                for lt in range(8):
                    ctr = pct.tile([128, 512], bf16, tag="pc", name=f"ctr{h}{lt}")
                    nc.tensor.matmul(
                        ctr[:, 0:65],
                        lhsT=ctxt[0:65, lt * 128:(lt + 1) * 128],
                        rhs=ident_bf[0:65, 0:65],
                        is_transpose=True,
                    )
                    zr = zr_pool.tile([128, 1], f32, tag="zr", name=f"zr{h}{lt}")
                    nc.vector.reciprocal(zr, ctr[:, 64:65])
                    nc.vector.tensor_tensor(
                        out_sb[:, lt, h * HD:(h + 1) * HD],
                        ctr[:, 0:HD],
                        zr.to_broadcast([128, HD]),
                        mult,
                    )
                    est["v"] += 64 * RATE["v"] + 2 * INIT["v"]
                    nc.sync.dma_start(
                        out.ap()[lt * 128:(lt + 1) * 128, h * HD:(h + 1) * HD],
                        out_sb[:, lt, h * HD:(h + 1) * HD],
                    )
                yield

        # ---- drive: A(0) ∥ V-proj, then B(hp) ∥ A(hp+1) ----
        def drain(g, n=None):
            cnt = 0
            while n is None or cnt < n:
                try:
                    next(g)
                except StopIteration:
                    return False
                cnt += 1
            return True

        vg = vproj_units()
        a_cur = phase_a(0)
        i = 0
        while True:
            alive_a = drain(a_cur, 1)
            if i % 4 == 0:
                drain(vg, 1)
            i += 1
            if not alive_a:
                break
        drain(vg)

        for hp in range(3):
            b = phase_b(hp)
            a_next = phase_a(hp + 1) if hp + 1 < 3 else None
            while True:
                alive_b = drain(b, 1)
                if a_next is not None:
                    drain(a_next, 2)
                if not alive_b:
                    break
            if a_next is not None:
                drain(a_next)

    nc.compile()
    return nc


def get_nc():
    if "nc" not in _nc_cache:
        _nc_cache["nc"] = _build_nc()
    return _nc_cache["nc"]


def make_in_maps(from_tensor, to_tensor, Wq, bq, Wk, bk, Wv, bv, dist_emb):
    E = np.asarray(dist_emb, np.float32)
    Epad = np.zeros((JP, HD), np.float32)
    Epad[: 2 * MAX_POS - 1] = E
    EFpad = np.zeros((JP, HD), np.float32)
    EFpad[: 2 * MAX_POS - 1] = E[::-1]
    ETd = np.ascontiguousarray(np.vstack([Epad.T, Epad.T]).astype(BF16))
    EFTd = np.ascontiguousarray(np.vstack([EFpad.T, EFpad.T]).astype(BF16))

    in_maps = []
    for c in range(NCORES):
        b = c // 2
        h0 = (c % 2) * HPC
        sl = slice(h0 * HD, (h0 + HPC) * HD)
        in_maps.append(
            {
                "xfT": np.ascontiguousarray(np.asarray(from_tensor[b], np.float32).T).astype(BF16),
                "xtT": np.ascontiguousarray(np.asarray(to_tensor[b], np.float32).T).astype(BF16),
                "wqT": np.ascontiguousarray(np.asarray(Wq, np.float32)[sl].T).astype(BF16),
                "wkT": np.ascontiguousarray(np.asarray(Wk, np.float32)[sl].T).astype(BF16),
                "wvT": np.ascontiguousarray(np.asarray(Wv, np.float32)[sl].T).astype(BF16),
                "bqp": np.ascontiguousarray(np.asarray(bq, np.float32)[sl].reshape(3, 128).T),
                "bkp": np.ascontiguousarray(np.asarray(bk, np.float32)[sl].reshape(3, 128).T),
                "bvr": np.asarray(bv, np.float32)[sl].reshape(1, HPC * HD).copy(),
                "ETd": ETd,
                "EFTd": EFTd,
            }
        )
    return in_maps


def assemble(results):
    full = np.zeros((B, L, H), np.float32)
    for c in range(NCORES):
        b = c // 2
        h0 = (c % 2) * HPC
        full[b, :, h0 * HD:(h0 + HPC) * HD] = results[c]["out"]
    return full


def kernel(**inputs):
    import os
    os.environ["BASS_NEVER_TRACE"] = "1"  # NTFF hook is absent in grading env
    in_maps = make_in_maps(**inputs)
    nc = get_nc()
    res = run_bass_kernel_spmd(nc, in_maps, core_ids=list(range(NCORES)))
    return assemble(res.results)


if __name__ == "__main__":
    rng = np.random.default_rng(0)
    ins = {
        "from_tensor": rng.standard_normal((B, L, H), dtype=np.float32),
        "to_tensor": rng.standard_normal((B, L, H), dtype=np.float32),
        "Wq": rng.standard_normal((H, H), dtype=np.float32) * 0.02,
        "bq": rng.standard_normal((H,), dtype=np.float32) * 0.02,
        "Wk": rng.standard_normal((H, H), dtype=np.float32) * 0.02,
        "bk": rng.standard_normal((H,), dtype=np.float32) * 0.02,
        "Wv": rng.standard_normal((H, H), dtype=np.float32) * 0.02,
        "bv": rng.standard_normal((H,), dtype=np.float32) * 0.02,
        "dist_emb": rng.standard_normal((2 * MAX_POS - 1, HD), dtype=np.float32) * 0.02,
    }
    out = kernel(**ins)
    print("ran", out.shape, out.dtype)
